# revision 7
# baseline (speedup 1.0000x reference)
"""MixHop GNN (2-layer, powers {0,1,2}) on 8 Trainium2 NeuronCores.

Strategy: nodes (rows of x / segment outputs) are sharded across the 8 cores;
edges are partitioned by destination node. Each SpMM phase processes 128-edge
tiles: source-feature rows arrive either pre-gathered from the host (layer-1
power-1, whose source table x is a kernel input) or via dma_gather from the
all-gathered halo tables. The per-tile selection matrices
S[e, d] = w_e * (dst_local[e] == d) are identical in all four phases, so the
host prebuilds them once in bf16 and the kernel streams them from DRAM
instead of rebuilding them on the DVE each phase. All matmul operands are
bf16 (f32 PSUM accumulate); tables and collectives are bf16, halving PE
time, gather DMA and AllGather bytes. Gather calls are grouped GB blocks at
a time to amortize the GpSimd descriptor-generation fixed cost. Matmuls are
reassociated through the (linear) propagation so features are propagated
post-weight where narrower:
  layer1: Y = A.x (256 wide), G1 = Y.W1_2 (128) -> AllGather -> P2 = A.G1
  layer2: V = h.[W2_1|W2_2] (80->pad 128) -> AllGather -> Q1 = A.V
          G3 = Q1[:,40:80] (pad 128)      -> AllGather -> Q2 = A.G3
The small per-power weight matrices are replicated on every core.
"""

import math
import numpy as np
import ml_dtypes
from contextlib import ExitStack
from dataclasses import dataclass

import concourse.bass as bass
import concourse.mybir as mybir
import concourse.tile as tile
from concourse import bacc
from concourse.bass_utils import run_bass_kernel_spmd

F32 = mybir.dt.float32
BF16 = mybir.dt.bfloat16
I16 = mybir.dt.int16
NPBF16 = ml_dtypes.bfloat16

NCORES = 8
SPLIT = 32768  # int16 gather-index limit
P = 128
GB = 7         # blocks per grouped gather call


@dataclass(frozen=True)
class Cfg:
    n: int
    npad: int
    nblk: int       # dst blocks per core
    f_in: int
    h: int
    c: int
    t_low: int      # gather tiles per block from low table
    t_high: int     # gather tiles per block from high table
    reg_low: tuple  # valid idx count per gather group, low call
    reg_high: tuple

    @property
    def rpc(self):
        return self.nblk * P

    @property
    def ngrp(self):
        return self.nblk // GB

    @property
    def tpb(self):
        return self.t_low + self.t_high

    @property
    def tpbs(self):  # tiles incl. the self-loop tile
        return self.tpb + 1


def preprocess(x, edge_index, nblk_pc=None):
    """Partition edges by dst block; build gather indices, bf16 S-tiles and
    pre-gathered layer-1 edge features."""
    n, f_in = x.shape
    if nblk_pc is None:
        nblk_pc = math.ceil(n / (NCORES * P))
    npad = NCORES * nblk_pc * P
    nblk_tot = NCORES * nblk_pc
    assert nblk_pc % GB == 0

    src = np.asarray(edge_index[0], dtype=np.int64)
    dst = np.asarray(edge_index[1], dtype=np.int64)

    # GCN norm with self loops (match reference fp32 math)
    deg = np.bincount(dst, minlength=npad).astype(np.float32)
    deg[:n] += 1.0
    with np.errstate(divide="ignore"):
        dinv = np.where(deg > 0, 1.0 / np.sqrt(deg), 0.0).astype(np.float32)
    w = (dinv[src] * dinv[dst]).astype(np.float32)
    wself = np.zeros(npad, np.float32)
    wself[:n] = dinv[:n] * dinv[:n]

    gblk = dst // P
    is_high = (src >= SPLIT).astype(np.int64)
    order = np.lexsort((src, is_high, gblk))
    src, w, gblk, is_high = (a[order] for a in (src, w, gblk, is_high))
    dst_local = (dst[order] % P).astype(np.int64)

    key = gblk * 2 + is_high
    cnt = np.bincount(key, minlength=nblk_tot * 2).reshape(nblk_tot, 2)
    # valid count per (block-slot, half): max across cores so num_idxs_reg is
    # a compile-time constant; shorter cores pad with dummy idx0/w0 edges
    cslot = cnt.reshape(NCORES, nblk_pc, 2)
    cmax = cslot.max(axis=0)                      # [nblk_pc, 2]
    t_low = int(math.ceil(cmax[:, 0].max() / P))
    t_high = int(math.ceil(cmax[:, 1].max() / P))
    if t_low:
        cmax[:, 0] = np.maximum(cmax[:, 0], 1)
    if t_high:
        cmax[:, 1] = np.maximum(cmax[:, 1], 1)
    tpb = t_low + t_high
    tpbs = tpb + 1
    assert t_low > 0 and t_high > 0

    starts = np.zeros(nblk_tot * 2, dtype=np.int64)
    starts[0::2] = 0
    starts[1::2] = t_low * P
    grp_first = np.searchsorted(key, np.arange(nblk_tot * 2), side="left")
    rank = np.arange(len(src)) - grp_first[key]
    slot = starts[key] + rank

    pad_src = np.zeros((nblk_tot, tpb * P), dtype=np.int64)   # 0 = safe row
    pad_dl = np.zeros((nblk_tot, tpb * P), dtype=np.int64)
    pad_w = np.zeros((nblk_tot, tpb * P), dtype=np.float32)
    pad_src[gblk, slot] = src
    pad_dl[gblk, slot] = dst_local
    pad_w[gblk, slot] = w
    # dummy high slots must map to idx 0 after -SPLIT
    pad_src[:, t_low * P:][pad_w[:, t_low * P:] == 0] = SPLIT

    # ---- S tiles (bf16): S[blk, e, t, d] = w * (dst_local == d) ----
    s3 = np.zeros((nblk_tot, tpb * P, P), dtype=NPBF16)
    s3[gblk, slot, dst_local] = w.astype(NPBF16)
    sself = np.zeros((nblk_tot, P, P), dtype=NPBF16)
    ar = np.arange(P)
    sself[:, ar, ar] = wself.reshape(nblk_tot, P).astype(NPBF16)
    s_all = np.concatenate(
        [s3.reshape(nblk_tot, tpb, P, P), sself[:, None]], axis=1)
    s_all = np.ascontiguousarray(s_all.transpose(0, 2, 1, 3)).reshape(
        NCORES, nblk_pc, P, tpbs * P)

    # ---- gather idx lists, grouped GB blocks per call ----
    # interior blocks fully padded with dummy idx 0; only the last block of a
    # group carries trailing -1 (ucode trims trailing negatives)
    iv_lo = pad_src[:, :t_low * P].astype(np.int16)
    iv_hi = (pad_src[:, t_low * P:] - SPLIT).astype(np.int16)
    ngrp = nblk_pc // GB
    last = (np.arange(nblk_tot) % nblk_pc) % GB == GB - 1
    cm_l = np.tile(cmax[:, 0], NCORES)
    cm_h = np.tile(cmax[:, 1], NCORES)
    mask_l = last[:, None] & (np.arange(t_low * P)[None, :] >= cm_l[:, None])
    mask_h = last[:, None] & (np.arange(t_high * P)[None, :] >= cm_h[:, None])
    iv_lo[mask_l] = -1
    iv_hi[mask_h] = -1
    reg_low = tuple(int((GB - 1) * t_low * P + cmax[g * GB + GB - 1, 0])
                    for g in range(ngrp))
    reg_high = tuple(int((GB - 1) * t_high * P + cmax[g * GB + GB - 1, 1])
                     for g in range(ngrp))

    def to_idx16(vals):  # [NCORES, ngrp, m*128] -> [NCORES, ngrp, 128, m*8]
        m = vals.shape[-1]
        a = vals.reshape(NCORES, ngrp, m // 16, 16).transpose(0, 1, 3, 2)
        return np.tile(a, (1, 1, 8, 1))

    iv_lo = iv_lo.reshape(NCORES, ngrp, GB * t_low * P)
    iv_hi = iv_hi.reshape(NCORES, ngrp, GB * t_high * P)
    idx16 = np.ascontiguousarray(np.concatenate(
        [to_idx16(iv_lo), to_idx16(iv_hi)], axis=3))

    # ---- pre-gathered layer-1 edge features (bf16) ----
    x_full = np.zeros((npad, f_in), dtype=np.float32)
    x_full[:n] = x
    xb16 = x_full.astype(NPBF16)
    xg = xb16[np.minimum(pad_src, npad - 1)]          # [nblk_tot, tpb*128, F]
    xg = xg.reshape(nblk_tot, tpb, P, f_in).transpose(0, 2, 1, 3)
    xg = np.ascontiguousarray(xg.reshape(NCORES, nblk_pc, P, tpb * f_in))

    # x slab per core, node-major and transposed chunk form
    fc = f_in // P
    xsl = np.ascontiguousarray(xb16.reshape(NCORES, nblk_pc * P, f_in))
    xT = xb16.reshape(NCORES, nblk_pc, P, fc, P).transpose(0, 1, 4, 3, 2)
    xT = np.ascontiguousarray(xT).reshape(NCORES, nblk_pc, P, fc * P)

    cfg = Cfg(n=n, npad=npad, nblk=nblk_pc, f_in=f_in, h=128, c=40,
              t_low=t_low, t_high=t_high, reg_low=reg_low, reg_high=reg_high)
    bundle = dict(s=s_all, idx16=idx16, xg=xg, xsl=xsl, xT=xT)
    return cfg, bundle


def build_nc(cfg: Cfg, num_devices=NCORES):
    nc = bacc.Bacc("TRN2", target_bir_lowering=False, debug=False,
                   num_devices=num_devices)
    F, H, C = cfg.f_in, cfg.h, cfg.c
    NB, RPC, NG = cfg.nblk, cfg.rpc, cfg.ngrp
    TPB, TPBS, TL, TH = cfg.tpb, cfg.tpbs, cfg.t_low, cfg.t_high
    FC = F // P

    # ---- I/O ----
    s_in = nc.dram_tensor("s", [NB, P, TPBS * P], BF16, kind="ExternalInput")
    xg_in = nc.dram_tensor("xg", [NB, P, TPB * F], BF16, kind="ExternalInput")
    xsl = nc.dram_tensor("xsl", [RPC, F], BF16, kind="ExternalInput")
    xT_in = nc.dram_tensor("xT", [NB, P, FC * P], BF16, kind="ExternalInput")
    idx16 = nc.dram_tensor("idx16", [NG, P, GB * TPB * 8], I16,
                           kind="ExternalInput")
    w10 = nc.dram_tensor("w10", [FC, P, H], BF16, kind="ExternalInput")
    w11 = nc.dram_tensor("w11", [FC, P, H], BF16, kind="ExternalInput")
    w12 = nc.dram_tensor("w12", [FC, P, H], BF16, kind="ExternalInput")
    w2a = nc.dram_tensor("w2a", [3, P, P], BF16, kind="ExternalInput")
    w2z = nc.dram_tensor("w2z", [3, P, C], BF16, kind="ExternalInput")
    b1t = nc.dram_tensor("b1t", [P, 3], F32, kind="ExternalInput")
    b2rep = nc.dram_tensor("b2rep", [P, 3 * C], F32, kind="ExternalInput")
    ident_in = nc.dram_tensor("ident", [P, P], BF16, kind="ExternalInput")
    y_out = nc.dram_tensor("y", [RPC, 3 * C], F32, kind="ExternalOutput")

    g1_loc = nc.dram_tensor("g1_loc", [RPC, H], BF16)
    g1_full = nc.dram_tensor("g1_full", [cfg.npad, H], BF16,
                             addr_space="Shared")
    v_loc = nc.dram_tensor("v_loc", [RPC, P], BF16)
    v_full = nc.dram_tensor("v_full", [cfg.npad, P], BF16, addr_space="Shared")
    g3_loc = nc.dram_tensor("g3_loc", [RPC, P], BF16)
    g3_full = nc.dram_tensor("g3_full", [cfg.npad, P], BF16,
                             addr_space="Shared")

    rg = [list(range(num_devices))]

    with tile.TileContext(nc) as tc, ExitStack() as top:
        cpool = top.enter_context(tc.tile_pool(name="const", bufs=1))
        perm = top.enter_context(tc.tile_pool(name="persist", bufs=1))

        ident = cpool.tile([P, P], BF16)
        nc.sync.dma_start(ident[:], ident_in[:, :])
        w10_sb = cpool.tile([P, FC, H], BF16)
        w11_sb = cpool.tile([P, FC, H], BF16)
        w12_sb = cpool.tile([P, FC, H], BF16)
        for c in range(FC):
            nc.sync.dma_start(w10_sb[:, c, :], w10[c])
            nc.sync.dma_start(w11_sb[:, c, :], w11[c])
            nc.sync.dma_start(w12_sb[:, c, :], w12[c])
        w2a_sb = cpool.tile([P, 3, P], BF16)
        w2z_sb = cpool.tile([P, 3, C], BF16)
        for c in range(3):
            nc.sync.dma_start(w2a_sb[:, c, :], w2a[c])
            nc.sync.dma_start(w2z_sb[:, c, :], w2z[c])
        b1_sb = cpool.tile([P, 3], F32)
        nc.sync.dma_start(b1_sb[:], b1t[:, :])
        b2_sb = cpool.tile([P, 3 * C], F32)
        nc.sync.dma_start(b2_sb[:], b2rep[:, :])

        pre1T = perm.tile([P, NB, 2, P], BF16)
        out0_sb = perm.tile([P, NB, C], F32)
        out1_sb = perm.tile([P, NB, C], F32)

        # shared gather double-buffers for phases 2-4 (elem = 128 in all
        # three), zeroed once so slots beyond num_idxs_reg can never inject
        # stale NaNs into w=0 matmul slots
        gbufs = []
        for nm in ("A", "B"):
            glow = perm.tile([P, GB * TL, P], BF16, name=f"glow{nm}")
            nc.vector.memset(glow[:], 0.0)
            ghigh = perm.tile([P, GB * TH, P], BF16, name=f"ghigh{nm}")
            nc.vector.memset(ghigh[:], 0.0)
            gbufs.append((glow, ghigh))

        def load_s(spool, b):
            # Activation-engine DMA queue: keeps per-block loads out of the
            # Sync queue so gather idx loads never stall behind them
            s_sb = spool.tile([P, TPBS, P], BF16, tag="s", name="s_sb")
            nc.scalar.dma_start(s_sb[:], s_in[b])
            return s_sb

        def phase_prefetch(ppool, loc):
            # all gather idx tiles + the local (self-loop) slab, issued at
            # phase start so nothing queues behind per-block traffic
            idx_all = ppool.tile([P, NG, GB * TPB * 8], I16, name="idx_all")
            for g in range(NG):
                nc.sync.dma_start(idx_all[:, g, :], idx16[g])
            gs_all = ppool.tile([P, NB, P], BF16, name="gs_all")
            for b in range(NB):
                nc.sync.dma_start(gs_all[:, b, :], loc[b * P:(b + 1) * P, :])
            return idx_all, gs_all

        # dedicated pre-loaded idx-count registers: a shared scratch register
        # would serialize each gather behind the previous one's DMA
        # completion (WAR on the register via tile's conservative sync)
        regs_low, regs_high = [], []
        for g in range(NG):
            rl = nc.gpsimd.alloc_register()
            nc.gpsimd.reg_mov(rl, cfg.reg_low[g])
            regs_low.append(rl)
            rh = nc.gpsimd.alloc_register()
            nc.gpsimd.reg_mov(rh, cfg.reg_high[g])
            regs_high.append(rh)

        def emit_gathers(idx_all, table, g):
            glow, ghigh = gbufs[g % 2]
            nc.gpsimd.dma_gather(
                out_ap=glow[:], in_ap=table[:, :],
                idxs_ap=idx_all[:, g, : GB * TL * 8],
                num_idxs=GB * TL * P, num_idxs_reg=regs_low[g],
                elem_size=P, single_packet=False)
            nc.gpsimd.dma_gather(
                out_ap=ghigh[:], in_ap=table[SPLIT:, :],
                idxs_ap=idx_all[:, g, GB * TL * 8:],
                num_idxs=GB * TH * P, num_idxs_reg=regs_high[g],
                elem_size=P, single_packet=False)
            return glow, ghigh

        # ------------- Phase 1: Y = A.x (pre-gathered) ; pre1 ; G1 ----------
        with ExitStack() as ph:
            gpool = ph.enter_context(tc.tile_pool(name="p1g", bufs=2))
            spool = ph.enter_context(tc.tile_pool(name="p1s", bufs=3))
            wpool = ph.enter_context(tc.tile_pool(name="p1w", bufs=2))
            pp_y = ph.enter_context(tc.tile_pool(name="p1y", bufs=2, space="PSUM"))
            pp_t = ph.enter_context(tc.tile_pool(name="p1t", bufs=2, space="PSUM"))
            pp_o = ph.enter_context(tc.tile_pool(name="p1o", bufs=1, space="PSUM"))
            pp_g1 = ph.enter_context(tc.tile_pool(name="p1g1", bufs=1, space="PSUM"))

            for b in range(NB):
                s_sb = load_s(spool, b)
                xg_sb = gpool.tile([P, TPB, F], BF16, tag="xg", name="xg_sb")
                nc.sync.dma_start(xg_sb[:], xg_in[b])
                xb = wpool.tile([P, F], BF16, tag="xb", name="xb")
                nc.sync.dma_start(xb[:], xsl[b * P:(b + 1) * P, :])
                xT_sb = wpool.tile([P, FC, P], BF16, tag="xT", name="xT_sb")
                nc.sync.dma_start(xT_sb[:], xT_in[b])

                y_ps = pp_y.tile([P, F], F32)
                for t in range(TPBS):
                    rhs = xg_sb[:, t, :] if t < TPB else xb[:]
                    nc.tensor.matmul(y_ps[:], lhsT=s_sb[:, t, :], rhs=rhs,
                                     start=(t == 0), stop=(t == TPBS - 1))

                y_sb = wpool.tile([P, F], BF16, tag="y", name="y_sb")
                nc.scalar.copy(y_sb[:], y_ps[:])
                t_ps = pp_t.tile([P, FC, P], BF16)
                for c in range(FC):
                    nc.tensor.transpose(t_ps[:, c, :],
                                        y_sb[:, c * P:(c + 1) * P], ident[:])
                yT = wpool.tile([P, FC, P], BF16, tag="yT", name="yT")
                nc.scalar.copy(yT[:], t_ps[:])

                o_ps = pp_o.tile([P, 2, P], F32)
                for c in range(FC):
                    nc.tensor.matmul(o_ps[:, 0, :], lhsT=w10_sb[:, c, :],
                                     rhs=xT_sb[:, c, :],
                                     start=(c == 0), stop=(c == FC - 1))
                for c in range(FC):
                    nc.tensor.matmul(o_ps[:, 1, :], lhsT=w11_sb[:, c, :],
                                     rhs=yT[:, c, :],
                                     start=(c == 0), stop=(c == FC - 1))
                nc.scalar.copy(pre1T[:, b, :, :], o_ps[:])

                g1_ps = pp_g1.tile([P, H], F32)
                for c in range(FC):
                    nc.tensor.matmul(g1_ps[:], lhsT=yT[:, c, :],
                                     rhs=w12_sb[:, c, :],
                                     start=(c == 0), stop=(c == FC - 1))
                g1_sb = wpool.tile([P, H], BF16, tag="g1", name="g1_sb")
                nc.vector.tensor_copy(g1_sb[:], g1_ps[:])
                nc.sync.dma_start(g1_loc[b * P:(b + 1) * P, :], g1_sb[:])

        nc.gpsimd.collective_compute(
            "AllGather", mybir.AluOpType.bypass, replica_groups=rg,
            ins=[g1_loc[:, :]], outs=[g1_full[:, :]])

        # ------------- Phase 2: P2 = A.G1 (transposed form) ; h ; V ; out0 --
        with ExitStack() as ph:
            spool = ph.enter_context(tc.tile_pool(name="p2s", bufs=4))
            ppool = ph.enter_context(tc.tile_pool(name="p2pf", bufs=1))
            wpool = ph.enter_context(tc.tile_pool(name="p2w", bufs=2))
            pp_p2 = ph.enter_context(tc.tile_pool(name="p2p", bufs=2, space="PSUM"))
            pp_v = ph.enter_context(tc.tile_pool(name="p2v", bufs=2, space="PSUM"))
            pp_o0 = ph.enter_context(tc.tile_pool(name="p2o", bufs=2, space="PSUM"))

            idx_all, gs_all = phase_prefetch(ppool, g1_loc)
            for g in range(NG):
                glow, ghigh = emit_gathers(idx_all, g1_full, g)

                for bi in range(GB):
                    b = g * GB + bi
                    s_sb = load_s(spool, b)

                    p2_ps = pp_p2.tile([P, P], F32)
                    for t in range(TPBS):
                        if t < TL:
                            lhs = glow[:, bi * TL + t, :]
                        elif t < TPB:
                            lhs = ghigh[:, bi * TH + t - TL, :]
                        else:
                            lhs = gs_all[:, b, :]
                        nc.tensor.matmul(p2_ps[:], lhsT=lhs, rhs=s_sb[:, t, :],
                                         start=(t == 0), stop=(t == TPBS - 1))

                    hT = wpool.tile([P, 3, P], BF16, tag="hT", name="hT")
                    for c in range(2):
                        nc.scalar.activation(
                            hT[:, c, :], pre1T[:, b, c, :],
                            mybir.ActivationFunctionType.Relu,
                            bias=b1_sb[:, c:c + 1])
                    nc.scalar.activation(
                        hT[:, 2, :], p2_ps[:],
                        mybir.ActivationFunctionType.Relu, bias=b1_sb[:, 2:3])

                    v_ps = pp_v.tile([P, P], F32)
                    o0_ps = pp_o0.tile([P, C], F32)
                    for c in range(3):
                        nc.tensor.matmul(v_ps[:], lhsT=hT[:, c, :],
                                         rhs=w2a_sb[:, c, :],
                                         start=(c == 0), stop=(c == 2))
                    for c in range(3):
                        nc.tensor.matmul(o0_ps[:], lhsT=hT[:, c, :],
                                         rhs=w2z_sb[:, c, :],
                                         start=(c == 0), stop=(c == 2))
                    v_sb = wpool.tile([P, P], BF16, tag="v", name="v_sb")
                    nc.vector.tensor_copy(v_sb[:], v_ps[:])
                    nc.sync.dma_start(v_loc[b * P:(b + 1) * P, :], v_sb[:])
                    nc.vector.tensor_copy(out0_sb[:, b, :], o0_ps[:])

        nc.gpsimd.collective_compute(
            "AllGather", mybir.AluOpType.bypass, replica_groups=rg,
            ins=[v_loc[:, :]], outs=[v_full[:, :]])

        # ------------- Phase 3: Q1 = A.V ; out1 ; G3 -------------
        with ExitStack() as ph:
            gpool = ph.enter_context(tc.tile_pool(name="p3g", bufs=1))
            spool = ph.enter_context(tc.tile_pool(name="p3s", bufs=4))
            ppool = ph.enter_context(tc.tile_pool(name="p3pf", bufs=1))
            pp_q1 = ph.enter_context(tc.tile_pool(name="p3q", bufs=2, space="PSUM"))

            # g3 rows: cols 0:40 real, 40:128 zeroed once per buffer
            g3bufs = []
            for nm in ("A", "B"):
                t_ = gpool.tile([P, P], BF16, name=f"g3{nm}")
                nc.vector.memset(t_[:], 0.0)
                g3bufs.append(t_)

            idx_all, gs_all = phase_prefetch(ppool, v_loc)
            for g in range(NG):
                glow, ghigh = emit_gathers(idx_all, v_full, g)

                for bi in range(GB):
                    b = g * GB + bi
                    s_sb = load_s(spool, b)

                    q1_ps = pp_q1.tile([P, 80], F32)
                    for t in range(TPBS):
                        if t < TL:
                            rhs = glow[:, bi * TL + t, 0:80]
                        elif t < TPB:
                            rhs = ghigh[:, bi * TH + t - TL, 0:80]
                        else:
                            rhs = gs_all[:, b, 0:80]
                        nc.tensor.matmul(q1_ps[:], lhsT=s_sb[:, t, :], rhs=rhs,
                                         start=(t == 0), stop=(t == TPBS - 1))

                    nc.vector.tensor_copy(out1_sb[:, b, :], q1_ps[:, 0:C])
                    g3_sb = g3bufs[b % 2]
                    nc.vector.tensor_copy(g3_sb[:, 0:C], q1_ps[:, C:2 * C])
                    nc.sync.dma_start(g3_loc[b * P:(b + 1) * P, :], g3_sb[:])

        nc.gpsimd.collective_compute(
            "AllGather", mybir.AluOpType.bypass, replica_groups=rg,
            ins=[g3_loc[:, :]], outs=[g3_full[:, :]])

        # ------------- Phase 4: Q2 = A.G3 ; logits ; log_softmax -------------
        with ExitStack() as ph:
            spool = ph.enter_context(tc.tile_pool(name="p4s", bufs=4))
            ppool = ph.enter_context(tc.tile_pool(name="p4pf", bufs=1))
            wpool = ph.enter_context(tc.tile_pool(name="p4w", bufs=3))
            pp_q2 = ph.enter_context(tc.tile_pool(name="p4q", bufs=2, space="PSUM"))

            idx_all, gs_all = phase_prefetch(ppool, g3_loc)
            for g in range(NG):
                glow, ghigh = emit_gathers(idx_all, g3_full, g)

                for bi in range(GB):
                    b = g * GB + bi
                    s_sb = load_s(spool, b)

                    q2_ps = pp_q2.tile([P, C], F32)
                    for t in range(TPBS):
                        if t < TL:
                            rhs = glow[:, bi * TL + t, 0:C]
                        elif t < TPB:
                            rhs = ghigh[:, bi * TH + t - TL, 0:C]
                        else:
                            rhs = gs_all[:, b, 0:C]
                        nc.tensor.matmul(q2_ps[:], lhsT=s_sb[:, t, :], rhs=rhs,
                                         start=(t == 0), stop=(t == TPBS - 1))

                    lg = wpool.tile([P, 3 * C], F32, tag="lg", name="lg")
                    nc.vector.tensor_add(lg[:, 0:C], out0_sb[:, b, :],
                                         b2_sb[:, 0:C])
                    nc.vector.tensor_add(lg[:, C:2 * C], out1_sb[:, b, :],
                                         b2_sb[:, C:2 * C])
                    nc.vector.tensor_add(lg[:, 2 * C:3 * C], q2_ps[:],
                                         b2_sb[:, 2 * C:3 * C])
                    negm = wpool.tile([P, 1], F32, tag="negm", name="negm")
                    nc.vector.tensor_reduce(negm[:], lg[:],
                                            axis=mybir.AxisListType.X,
                                            op=mybir.AluOpType.max, negate=True)
                    e = wpool.tile([P, 3 * C], F32, tag="e", name="e")
                    s = wpool.tile([P, 1], F32, tag="sm", name="sm")
                    nc.scalar.activation(e[:], lg[:],
                                         mybir.ActivationFunctionType.Exp,
                                         bias=negm[:, 0:1], accum_out=s[:])
                    ls = wpool.tile([P, 1], F32, tag="ls", name="ls")
                    nc.scalar.activation(ls[:], s[:],
                                         mybir.ActivationFunctionType.Ln)
                    yb = wpool.tile([P, 3 * C], F32, tag="yb", name="yb")
                    nc.vector.tensor_scalar(
                        out=yb[:], in0=lg[:], scalar1=negm[:, 0:1],
                        scalar2=ls[:, 0:1],
                        op0=mybir.AluOpType.add, op1=mybir.AluOpType.subtract)
                    nc.sync.dma_start(y_out[b * P:(b + 1) * P, :], yb[:])

    nc.compile()
    return nc


_CACHE = {}


def _get_nc(cfg):
    if cfg not in _CACHE:
        _CACHE[cfg] = build_nc(cfg)
    return _CACHE[cfg]


def make_inputs(cfg, bundle, inputs):
    F, H, C = cfg.f_in, cfg.h, cfg.c
    FC = F // P
    W10 = np.ascontiguousarray(
        np.asarray(inputs["W1_0"], np.float32).reshape(FC, P, H)).astype(NPBF16)
    W11 = np.ascontiguousarray(
        np.asarray(inputs["W1_1"], np.float32).reshape(FC, P, H)).astype(NPBF16)
    W12 = np.ascontiguousarray(
        np.asarray(inputs["W1_2"], np.float32).reshape(FC, P, H)).astype(NPBF16)
    w2a = np.zeros((3, P, P), np.float32)
    w2a[:, :, 0:C] = np.asarray(inputs["W2_1"], np.float32).reshape(3, P, C)
    w2a[:, :, C:2 * C] = np.asarray(inputs["W2_2"], np.float32).reshape(3, P, C)
    w2a = w2a.astype(NPBF16)
    w2z = np.ascontiguousarray(
        np.asarray(inputs["W2_0"], np.float32).reshape(3, P, C)).astype(NPBF16)
    b1t = np.ascontiguousarray(
        np.asarray(inputs["b1"], np.float32).reshape(3, P).T)
    b2rep = np.ascontiguousarray(
        np.tile(np.asarray(inputs["b2"], np.float32)[None, :], (P, 1)))
    ident = np.eye(P, dtype=NPBF16)

    in_maps = []
    for i in range(NCORES):
        in_maps.append({
            "s": bundle["s"][i],
            "xg": bundle["xg"][i],
            "xsl": bundle["xsl"][i],
            "xT": bundle["xT"][i],
            "idx16": bundle["idx16"][i],
            "w10": W10, "w11": W11, "w12": W12,
            "w2a": w2a, "w2z": w2z,
            "b1t": b1t, "b2rep": b2rep,
            "ident": ident,
        })
    return in_maps


def kernel(**inputs):
    x = np.asarray(inputs["x"], np.float32)
    edge_index = np.asarray(inputs["edge_index"])
    cfg, bundle = preprocess(x, edge_index)
    nc = _get_nc(cfg)
    in_maps = make_inputs(cfg, bundle, inputs)
    res = run_bass_kernel_spmd(nc, in_maps, core_ids=list(range(NCORES)))
    y = np.concatenate([res.results[i]["y"] for i in range(NCORES)], axis=0)
    return y[:cfg.n]
